# revision 20
# baseline (speedup 1.0000x reference)
"""GCN (2-layer) + mean-pool + MLP head on 8 TRN2 NeuronCores.

Strategy (dst-sharded graph partitioning):
- Nodes sharded 8 ways; core c owns nodes [c*NLOC, (c+1)*NLOC) and all edges
  whose dst lands in its shard.
- GCN normalization factorizes: out[v] = dis[v]*(sum_e dis[src]*h[src] +
  dis[v]*h[v]) + b with dis = 1/sqrt(deg+1), so messages are gathered from a
  raw (unscaled) bf16 feature table and the full norm dis_src*dis_dst is baked
  into host-precomputed weighted one-hot tiles streamed from DRAM.
- Edge aggregation: edges sorted by (src-group, dst-chunk); dma_gather (the
  only fast indirect path on this HW, ~9ns/row, Q7-bound) pulls 128-row
  message tiles; PE contracts onehot^T @ msgs into per-chunk PSUM, drained to
  an SBUF f32 accumulator. int16 gather indices force 4 source groups of
  N/4 table rows; segments are padded to the max count across cores and split
  into per-tile "pieces", each with its own one-hot.
- Self-loops never touch the gather path: per-chunk diag(dis^2) matmuls on
  locally available features (L1: own x rows shipped per-core; L2: own relu1
  kept in SBUF).
- Band-major table layout (node (r,i) -> row (i//BND)*GRP + r*BND + i%BND)
  makes AllGather stage b fill exactly source group b, so 4 staged AllGathers
  (separate DRAM tensors for precise dependency tracking) pipeline with L1's
  transform, and L2 group-g gathers are gated only on stage g. x is
  pre-permuted host-side so both layers share one edge dataset.
- Per-chunk transform: cast, PE-transpose, matmul with W (+bias via a rank-1
  ones matmul), relu. Mean-pool via precomputed batch one-hot matmuls into
  per-core partials + AllReduce; MLP head computed redundantly on every core;
  core 0's output returned.
"""
import sys
sys.path.insert(0, '/opt/trn_rl_repo')
import contextlib
import numpy as np
import ml_dtypes

import concourse.bass as bass
import concourse.bacc as bacc
import concourse.mybir as mybir
import concourse.tile as tile
from concourse import library_config
from concourse.bass_utils import run_bass_kernel_spmd

BF16 = ml_dtypes.bfloat16
CORES = 8
F = 128          # feature/hidden width (fixed at 128 = partition width)
NGRP = 4         # src groups (int16 gather index limit)
CALL_TILES = 80  # tiles (of 128 rows) per dma_gather call


class Geom:
    def __init__(self, n_nodes=100000, n_edges=1600000, n_graphs=64, a_dim=8):
        assert n_nodes % (CORES * NGRP) == 0 or True
        self.N = n_nodes
        self.E = n_edges
        self.G = n_graphs
        self.A = a_dim
        self.NLOC = n_nodes // CORES
        self.GRP = n_nodes // NGRP
        assert self.GRP <= 32767, "int16 gather index limit"
        self.CH = (self.NLOC + 127) // 128  # dst chunks per core


def _prep(geom, x, edge_index, batch, W1, b1, W2, b2, fc1_w, fc1_b, fc2_w, fc2_b):
    """Host-side preprocessing: degrees, edge sharding/sorting, padding plan,
    per-core input arrays."""
    g_ = geom
    N, NLOC, GRP, CH = g_.N, g_.NLOC, g_.GRP, g_.CH
    src = np.asarray(edge_index[0], dtype=np.int64)
    dst = np.asarray(edge_index[1], dtype=np.int64)
    batch = np.asarray(batch, dtype=np.int64)

    deg = np.bincount(dst, minlength=N).astype(np.float32) + 1.0
    dis = (1.0 / np.sqrt(deg)).astype(np.float32)

    assert NLOC % NGRP == 0
    BND = NLOC // NGRP
    # band-major table layout: node u=(r,i) -> row (i//BND)*GRP + r*BND + i%BND
    # so AllGather stage b fills exactly table rows [b*GRP,(b+1)*GRP) = group b
    u = np.arange(N, dtype=np.int64)
    r_, i_ = u // NLOC, u % NLOC
    row_of_node = (i_ // BND) * GRP + r_ * BND + (i_ % BND)
    node_of_row = np.empty(N, np.int64)
    node_of_row[row_of_node] = u
    # (row_of_node is recomputed after balancing permutations below)

    core_of = dst // NLOC
    per_core = []
    core_posn = []  # per core: local node -> position (chunk*128+slot)
    cnt = np.zeros((CORES, NGRP * CH), np.int64)
    vloc = np.arange(NLOC, dtype=np.int64)
    for c in range(CORES):
        m = core_of == c
        s = src[m]
        d_raw = dst[m] - c * NLOC
        w = (dis[s] * dis[dst[m]]).astype(np.float32)  # dis_src*dis_dst
        s = row_of_node[s]  # table rows, band-major
        sg = s // GRP

        # balance per-(group,chunk) edge counts across cores by permuting
        # local nodes WITHIN their AllGather band (keeps src groups fixed)
        dvec = np.zeros((NLOC, NGRP), np.int64)
        np.add.at(dvec, (d_raw, sg), 1)
        posn = np.empty(NLOC, np.int64)
        Lb = np.zeros((CH, NGRP), np.float64)
        for b in range(NGRP):
            lo_n, hi_n = b * BND, (b + 1) * BND
            nodes = np.arange(lo_n, hi_n)
            nodes = nodes[np.argsort(-dvec[nodes].sum(1), kind='stable')]
            ch_lo, ch_hi = lo_n // 128, (hi_n - 1) // 128
            chs = np.arange(ch_lo, ch_hi + 1)
            cap = np.minimum((chs + 1) * 128, hi_n) - np.maximum(chs * 128, lo_n)
            nxt = np.maximum(chs * 128, lo_n).astype(np.int64)  # next free position
            left = cap.copy()
            for v in nodes:
                dots = Lb[chs] @ dvec[v]
                dots[left <= 0] = np.inf
                j = int(np.argmin(dots))
                posn[v] = nxt[j]
                nxt[j] += 1
                left[j] -= 1
                Lb[chs[j]] += dvec[v]
        core_posn.append(posn)

        d = posn[d_raw]
        ch = d >> 7
        sl = (d & 127).astype(np.int64)
        seg = sg * CH + ch
        order = np.argsort(seg, kind='stable')
        per_core.append((s[order], seg[order], sl[order], w[order]))
        cnt[c] = np.bincount(seg, minlength=NGRP * CH)

    # fold the balancing permutations into the table-row map; bands are
    # preserved by construction so src groups (seg) are unchanged
    posn_all = np.concatenate(core_posn)  # [N] local position per node
    i2 = posn_all[u % NLOC * 0 + np.arange(N) % NLOC]  # placeholder, replaced below
    i2 = np.concatenate([core_posn[c] for c in range(CORES)])
    row_of_node = (i2 // BND) * GRP + (u // NLOC) * BND + (i2 % BND)
    node_of_row = np.full(N, -1, np.int64)
    node_of_row[row_of_node] = u
    # remap each core's src table rows with the final permutation-aware map
    for c in range(CORES):
        s, seg, sl, w = per_core[c]
        m = core_of == c
        s_nodes = src[m]
        d_raw = dst[m] - c * NLOC
        sg = row_of_node[s_nodes] // GRP
        d = core_posn[c][d_raw]
        ch = d >> 7
        sl2 = (d & 127).astype(np.int64)
        seg2 = sg * CH + ch
        order = np.argsort(seg2, kind='stable')
        w2 = (dis[s_nodes] * dis[dst[m]]).astype(np.float32)
        per_core[c] = (row_of_node[s_nodes][order], seg2[order], sl2[order], w2[order])
        cnt[c] = np.bincount(seg2, minlength=NGRP * CH)

    L = cnt.max(axis=0)  # per-segment padded length = max count across cores
    # group streams rounded up to x128 via a tail filler segment
    base = np.zeros(NGRP * CH + 1, np.int64)
    grp_len = []
    off = 0
    for gidx in range(NGRP):
        for ch in range(CH):
            base[gidx * CH + ch] = off
            off += int(L[gidx * CH + ch])
        if off % 128:
            off += 128 - off % 128  # group tail pad (no pieces)
        grp_len.append(off - (int(base[gidx * CH])))
    base[-1] = off
    S_total = off
    grp_tiles = [gl // 128 for gl in grp_len]
    # fix base sentinel per group end (used only for slicing idx streams)
    grp_lo = [int(base[g * CH]) for g in range(NGRP)]

    # piece-level metadata (identical across cores): a piece is the part of a
    # segment falling in one 128-row tile
    piece_tile = []   # tile index within the group
    piece_chunk = []
    piece_first = []
    piece_last = []
    pieces_by_grp = []
    for gidx in range(NGRP):
        plist = []
        for ch in range(CH):
            segi = gidx * CH + ch
            lo = int(base[segi]) - grp_lo[gidx]
            hi = lo + int(L[segi])
            if hi == lo:
                continue
            tlo, thi = lo // 128, (hi - 1) // 128
            for t in range(tlo, thi + 1):
                plist.append((t, ch, t == tlo, t == thi))
        pieces_by_grp.append(plist)
        for (t, ch, fi, la) in plist:
            piece_tile.append(t)
            piece_chunk.append(ch)
            piece_first.append(fi)
            piece_last.append(la)
    NP_TOT = len(piece_tile)
    TT = S_total // 128

    # call plan per group: list of (ntiles, npieces); pieces are consumed in
    # order and each call covers the pieces whose tile is inside the call
    call_plan = []
    for gidx in range(NGRP):
        plist = pieces_by_grp[gidx]
        calls = []
        t0 = 0
        pi = 0
        left = grp_tiles[gidx]
        while left > 0:
            take = min(CALL_TILES, left)
            np_call = 0
            while pi < len(plist) and plist[pi][0] < t0 + take:
                np_call += 1
                pi += 1
            calls.append((take, np_call))
            t0 += take
            left -= take
        assert pi == len(plist)
        call_plan.append(calls)

    # per-core streams
    in_maps = []
    counts = np.bincount(batch, minlength=g_.G).astype(np.float32)
    invc = (1.0 / np.maximum(counts, 1.0)).astype(np.float32).reshape(g_.G, 1)
    xt = np.asarray(x, dtype=np.float32).astype(BF16)[node_of_row]  # band-major rows
    pad_nodes = CH * 128 - NLOC
    for c in range(CORES):
        s, seg, sl, w = per_core[c]
        # destination positions in padded stream
        seg_start_in_sorted = np.searchsorted(seg, np.arange(NGRP * CH))
        rank = np.arange(len(seg)) - seg_start_in_sorted[seg]
        pos = base[seg] + rank
        idxv = np.zeros(S_total, np.int16)
        idxv[pos] = (s - (s // GRP) * GRP).astype(np.int16)
        im = {}
        for gidx in range(NGRP):
            lo = grp_lo[gidx]
            hi = lo + grp_len[gidx]
            seg16 = idxv[lo:hi].reshape(-1, 16).T  # [16, n/16]
            im[f"idxg{gidx}"] = np.tile(seg16, (8, 1)).copy()
        # per-piece weighted one-hot tiles: A[p, e, d] = norm weight.
        # piece id for an edge: map (segment, tile-of-pos) -> piece index
        pk = {}
        for p, (gidx_, t_, ch_, fi_, la_) in enumerate(
                [(gg, t, ch, fi, la) for gg in range(NGRP)
                 for (t, ch, fi, la) in pieces_by_grp[gg]]):
            pk[(gidx_, t_, ch_)] = p
        e_g = seg // CH
        e_ch = seg % CH
        e_t = np.empty(len(pos), np.int64)
        for gidx in range(NGRP):
            m2 = e_g == gidx
            e_t[m2] = (pos[m2] - grp_lo[gidx]) // 128
        e_p = np.array([pk[(gg, tt_, cc)] for gg, tt_, cc in
                        zip(e_g, e_t, e_ch)], np.int64)
        A = np.zeros(NP_TOT * 128 * 128, BF16)
        e_slot = np.empty(len(pos), np.int64)
        for gidx in range(NGRP):
            m2 = e_g == gidx
            e_slot[m2] = (pos[m2] - grp_lo[gidx]) % 128
        A[e_p * (128 * 128) + e_slot * 128 + sl] = w.astype(BF16)
        im["oh"] = np.ascontiguousarray(
            A.reshape(NP_TOT, 128, 128).transpose(1, 0, 2).reshape(128, NP_TOT * 128))
        # precomputed batch one-hots: B[ch, node_slot, graph]
        B = np.zeros(CH * 128 * g_.G, BF16)
        bl = batch[c * NLOC:(c + 1) * NLOC]
        B[core_posn[c] * g_.G + bl] = np.float32(1.0)
        im["ohb"] = np.ascontiguousarray(
            B.reshape(CH, 128, g_.G).transpose(1, 0, 2).reshape(128, CH * g_.G))
        im["invc"] = invc
        im["xt"] = xt
        im["ident"] = np.eye(128, dtype=np.float32).astype(BF16)
        # self-loop handling without gathers: per-chunk diag(dis^2) + own x rows
        posn = core_posn[c]
        dis2 = np.zeros(CH * 128, np.float32)
        dis2[posn] = dis[c * NLOC:(c + 1) * NLOC] ** 2
        S2 = np.zeros((128, CH * 128), np.float32)
        nn = np.arange(CH * 128)
        S2[nn % 128, nn] = dis2
        im["selfoh"] = S2.astype(BF16)
        xs_loc = np.zeros((CH * 128, F), BF16)
        xs_loc[posn] = np.asarray(x, np.float32)[c * NLOC:(c + 1) * NLOC].astype(BF16)
        im["xself"] = xs_loc
        im["w1"] = np.asarray(W1, np.float32).astype(BF16)
        im["w2"] = np.asarray(W2, np.float32).astype(BF16)
        im["fc1w"] = np.asarray(fc1_w, np.float32).astype(BF16)
        im["fc2w"] = np.asarray(fc2_w, np.float32).astype(BF16)
        im["b1"] = np.asarray(b1, np.float32).astype(BF16).reshape(1, F)
        im["b2"] = np.asarray(b2, np.float32).astype(BF16).reshape(1, F)
        im["fc2b"] = np.asarray(fc2_b, np.float32).astype(BF16).reshape(1, g_.A)
        im["fc1b"] = np.asarray(fc1_b, np.float32).reshape(F, 1).copy()
        in_maps.append(im)

    plan = dict(
        TT=TT, NP_TOT=NP_TOT, grp_tiles=grp_tiles, call_plan=call_plan,
        piece_tile=piece_tile, piece_chunk=piece_chunk,
        piece_first=piece_first, piece_last=piece_last,
    )
    return plan, in_maps


def _build(geom, plan, tag="", stages="all"):
    g_ = geom
    N, NLOC, GRP, CH, G, A = g_.N, g_.NLOC, g_.GRP, g_.CH, g_.G, g_.A
    TT = plan["TT"]
    bf = mybir.dt.bfloat16
    f32 = mybir.dt.float32
    AL = mybir.AluOpType
    ACT = mybir.ActivationFunctionType

    nc = bacc.Bacc("TRN2", debug=False, target_bir_lowering=False)
    P = {}
    def par(name, shape, dt):
        P[name] = nc.declare_dram_parameter(name + tag, list(shape), dt, isOutput=False)
        return P[name]

    for gidx in range(NGRP):
        par(f"idxg{gidx}", [128, plan["grp_tiles"][gidx] * 8], mybir.dt.int16)
    par("oh", [128, plan["NP_TOT"] * 128], bf)
    par("ohb", [128, CH * G], bf)
    par("invc", [G, 1], f32)
    par("xt", [N, F], bf)
    par("ident", [128, 128], bf)
    par("selfoh", [128, CH * 128], bf)
    par("xself", [CH * 128, F], bf)
    par("w1", [F, F], bf)
    par("w2", [F, F], bf)
    par("fc1w", [F, F], bf)
    par("fc2w", [F, A], bf)
    par("b1", [1, F], bf)
    par("b2", [1, F], bf)
    par("fc2b", [1, A], bf)
    par("fc1b", [F, 1], f32)
    out_ext = nc.declare_dram_parameter("out" + tag, [G, A], f32, isOutput=True)

    BND = NLOC // NGRP
    agin = [nc.dram_tensor(f"agin{b}" + tag, [BND, F], bf) for b in range(NGRP)]
    tbl2 = [nc.dram_tensor(f"tbl2{b}" + tag, [GRP, F], bf, addr_space="Shared")
            for b in range(NGRP)]
    ar_in = nc.dram_tensor("arin" + tag, [G, F], f32)
    ar_out = nc.dram_tensor("arout" + tag, [G, F], f32, addr_space="Shared")

    with tile.TileContext(nc) as tc:
        with contextlib.ExitStack() as ex:
            pc = ex.enter_context(tc.tile_pool(name="const", bufs=1))
            pacc_pool = ex.enter_context(tc.tile_pool(name="accp", bufs=1))
            pidx = ex.enter_context(tc.tile_pool(name="idx", bufs=2))
            pg = ex.enter_context(tc.tile_pool(name="gbuf", bufs=2))
            poh = ex.enter_context(tc.tile_pool(name="oh", bufs=2))
            ptf = ex.enter_context(tc.tile_pool(name="tf", bufs=3))
            pseg = ex.enter_context(tc.tile_pool(name="pseg", bufs=2, space=bass.MemorySpace.PSUM))
            ptp = ex.enter_context(tc.tile_pool(name="ptp", bufs=6, space=bass.MemorySpace.PSUM))

            nc.gpsimd.load_library(library_config.mlp)

            # load constants
            ct = {}
            for nm in ["ohb", "ident",
                       "w1", "w2", "fc1w", "fc2w", "b1", "b2", "fc2b", "fc1b", "invc"]:
                t = pc.tile([P[nm].shape[0], P[nm].shape[1]], P[nm].dtype, tag=nm)
                nc.sync.dma_start(out=t[:], in_=P[nm][:, :])
                ct[nm] = t
            ones = pc.tile([1, 128], bf)
            nc.vector.memset(ones[:], 1.0)

            acc = pacc_pool.tile([128, CH * 128], f32)
            pacc = pacc_pool.tile([G, F], f32)
            rel1k = pacc_pool.tile([128, CH * 128], bf)


            layers = () if stages == "noop" else ((1,) if stages in ("edge1", "l1", "l1noag") else (1, 2))
            for layer in layers:
                wt = ct["w1"] if layer == 1 else ct["w2"]
                bt = ct["b1"] if layer == 1 else ct["b2"]

                # -------- edge phase --------
                p_global = 0
                ps = None
                for gidx in range(NGRP):
                    pos16 = 0  # column offset into idxg (16-wrapped)
                    t0call = 0
                    for (ntiles, npieces) in plan["call_plan"][gidx]:
                        nidx = ntiles * 128
                        idx_t = pidx.tile([128, nidx // 16], mybir.dt.int16)
                        nc.sync.dma_start(
                            out=idx_t[:],
                            in_=P[f"idxg{gidx}"][:, pos16:pos16 + nidx // 16])
                        gbuf = pg.tile([128, ntiles, F], bf)
                        srcap = (P["xt"].ap()[gidx * GRP:(gidx + 1) * GRP]
                                 if layer == 1 else tbl2[gidx].ap())
                        nc.gpsimd.dma_gather(
                            gbuf[:], srcap,
                            idx_t[:], nidx, nidx, F, single_packet=False)
                        ohslab = poh.tile([128, max(npieces, 1), 128], bf)
                        if npieces:
                            nc.sync.dma_start(
                                out=ohslab[:, :npieces, :],
                                in_=P["oh"].ap().rearrange("p (t d) -> p t d", d=128)[:, p_global:p_global + npieces, :])
                        for pp in range(npieces):
                            p = p_global + pp
                            ch = int(plan["piece_chunk"][p])
                            tloc = int(plan["piece_tile"][p]) - t0call
                            if plan["piece_first"][p]:
                                ps = pseg.tile([128, 128], f32)
                            nc.tensor.matmul(ps[:], ohslab[:, pp, :],
                                             gbuf[:, tloc, :],
                                             start=bool(plan["piece_first"][p]),
                                             stop=bool(plan["piece_last"][p]))
                            if plan["piece_last"][p]:
                                csl = acc[:, ch * 128:(ch + 1) * 128]
                                if gidx == 0:
                                    nc.vector.tensor_copy(csl, ps[:])
                                else:
                                    nc.vector.tensor_tensor(csl, csl, ps[:], AL.add)
                        p_global += npieces
                        t0call += ntiles
                        pos16 += nidx // 16

                # -------- transform phase --------
                ag_next = 0
                for ch in (range(CH) if stages != "edge1" else range(0)):
                    rows = min(128, NLOC - ch * 128)
                    # self-loop term: diag(dis^2) @ own features (no gather)
                    so = ptf.tile([128, 128], bf, tag="so")
                    nc.sync.dma_start(out=so[:], in_=P["selfoh"][:, ch * 128:(ch + 1) * 128])
                    if layer == 1:
                        xs = ptf.tile([128, 128], bf, tag="xs")
                        nc.sync.dma_start(out=xs[:], in_=P["xself"][ch * 128:(ch + 1) * 128, :])
                        selfrhs = xs[:]
                    else:
                        selfrhs = rel1k[:, ch * 128:(ch + 1) * 128]
                    ps2 = ptp.tile([128, 128], f32, tag="ps")
                    nc.tensor.matmul(ps2[:], so[:], selfrhs, start=True, stop=True)
                    csl2 = acc[:, ch * 128:(ch + 1) * 128]
                    nc.vector.tensor_tensor(csl2, csl2, ps2[:], AL.add)
                    aggS = ptf.tile([128, 128], bf)
                    nc.vector.tensor_copy(aggS[:], acc[:, ch * 128:(ch + 1) * 128])
                    psT = ptp.tile([128, 128], bf, tag="ps")
                    nc.tensor.transpose(psT[:], aggS[:], ct["ident"][:])
                    aggT = ptf.tile([128, 128], bf)
                    nc.scalar.copy(aggT[:], psT[:])
                    psO = ptp.tile([128, 128], f32, tag="ps")
                    nc.tensor.matmul(psO[:], aggT[:], wt[:], start=True, stop=False)
                    nc.tensor.matmul(psO[:], ones[:1, :], bt[:1, :], start=False, stop=True)
                    if layer == 1:
                        rel = rel1k[:, ch * 128:(ch + 1) * 128]
                    else:
                        rel_t = ptf.tile([128, 128], bf, tag="rel")
                        rel = rel_t[:]
                    nc.scalar.activation(rel, psO[:], ACT.Relu)
                    if layer == 1:
                        lo = ch * 128
                        hi = lo + rows
                        b0, b1 = lo // BND, (hi - 1) // BND
                        for b in range(b0, b1 + 1):
                            s0, s1 = max(lo, b * BND), min(hi, (b + 1) * BND)
                            nc.sync.dma_start(
                                out=agin[b][s0 - b * BND:s1 - b * BND, :],
                                in_=rel[s0 - lo:s1 - lo, :])
                        if stages not in ("edge1", "l1noag"):
                            while ag_next < NGRP and (ag_next + 1) * BND <= hi:
                                nc.gpsimd.collective_compute(
                                    "AllGather", AL.bypass,
                                    ins=[agin[ag_next].ap().opt()],
                                    outs=[tbl2[ag_next].ap().opt()],
                                    replica_groups=[list(range(CORES))])
                                ag_next += 1
                    else:
                        psB = ptp.tile([G, F], f32, tag="ps")
                        nc.tensor.matmul(psB[:], ct["ohb"][:, ch * G:(ch + 1) * G],
                                         rel, start=True, stop=True)
                        if ch == 0:
                            nc.vector.tensor_copy(pacc[:], psB[:])
                        else:
                            nc.vector.tensor_tensor(pacc[:], pacc[:], psB[:], AL.add)

                if layer == 1 and stages not in ("edge1", "l1noag"):
                    while ag_next < NGRP:
                        nc.gpsimd.collective_compute(
                            "AllGather", AL.bypass,
                            ins=[agin[ag_next].ap().opt()],
                            outs=[tbl2[ag_next].ap().opt()],
                            replica_groups=[list(range(CORES))])
                        ag_next += 1

            # -------- pooling + head --------
            if stages in ("edge1", "l1", "l1noag", "noop"):
                z0 = ptf.tile([G, A], f32)
                nc.vector.memset(z0[:], 0.0)
                nc.sync.dma_start(out=out_ext[:, :], in_=z0[:])
            else:
                nc.sync.dma_start(out=ar_in[:, :], in_=pacc[:])
                nc.gpsimd.collective_compute(
                    "AllReduce", AL.add,
                    ins=[ar_in.ap().opt()], outs=[ar_out.ap().opt()],
                    replica_groups=[list(range(CORES))])
                pooledf = ptf.tile([G, F], f32)
                nc.sync.dma_start(out=pooledf[:], in_=ar_out[:, :])
                pooledb = ptf.tile([G, F], bf)
                nc.vector.tensor_scalar(pooledb[:], pooledf[:], ct["invc"][:, :1],
                                        None, AL.mult)
                psPT = ptp.tile([F, G], bf, tag="ps")
                nc.tensor.transpose(psPT[:], pooledb[:], ct["ident"][:G, :G])
                pooledT = ptf.tile([F, G], bf)
                nc.scalar.copy(pooledT[:], psPT[:])
                psZ = ptp.tile([F, G], f32, tag="ps")
                nc.tensor.matmul(psZ[:], ct["fc1w"][:], pooledT[:], start=True, stop=True)
                zT = ptf.tile([F, G], bf)
                nc.scalar.activation(zT[:], psZ[:], ACT.Relu, bias=ct["fc1b"][:, :1])
                psO2 = ptp.tile([G, A], f32, tag="ps")
                nc.tensor.matmul(psO2[:], zT[:], ct["fc2w"][:], start=True, stop=False)
                nc.tensor.matmul(psO2[:], ones[:1, :G], ct["fc2b"][:1, :],
                                 start=False, stop=True)
                outt = ptf.tile([G, A], f32)
                nc.scalar.activation(outt[:], psO2[:], ACT.Sigmoid)
                nc.sync.dma_start(out=out_ext[:, :], in_=outt[:])

    nc.compile()
    return nc


_GEOM = Geom()
_CALLS = [0]


def kernel(x, edge_index, batch, W1, b1, W2, b2, fc1_w, fc1_b, fc2_w, fc2_b):
    plan, in_maps = _prep(_GEOM, x, edge_index, batch, W1, b1, W2, b2,
                          fc1_w, fc1_b, fc2_w, fc2_b)
    tag = f"_k{_CALLS[0]}" if _CALLS[0] else ""
    _CALLS[0] += 1
    nc = _build(_GEOM, plan, tag=tag)
    res = run_bass_kernel_spmd(nc, [{k + tag: v for k, v in m.items()} for m in in_maps],
                               list(range(CORES)))
    return np.asarray(res.results[0]["out" if not tag else "out" + tag],
                      dtype=np.float32)


# revision 21
# speedup vs baseline: 1.4023x; 1.4023x over previous
"""GCN (2-layer) + mean-pool + MLP head on 8 TRN2 NeuronCores.

Strategy (dst-sharded graph partitioning):
- Nodes sharded 8 ways; core c owns nodes [c*NLOC, (c+1)*NLOC) and all edges
  whose dst lands in its shard.
- GCN normalization factorizes: out[v] = dis[v]*(sum_e dis[src]*h[src] +
  dis[v]*h[v]) + b with dis = 1/sqrt(deg+1), so messages are gathered from a
  raw (unscaled) bf16 feature table and the full norm dis_src*dis_dst is baked
  into host-precomputed weighted one-hot tiles streamed from DRAM.
- Edge aggregation: edges sorted by (src-group, dst-chunk); dma_gather (the
  only fast indirect path on this HW, ~9ns/row, Q7-bound) pulls 128-row
  message tiles; PE contracts onehot^T @ msgs into per-chunk PSUM, drained to
  an SBUF f32 accumulator. int16 gather indices force 4 source groups of
  N/4 table rows; segments are padded to the max count across cores and split
  into per-tile "pieces", each with its own one-hot.
- Self-loops never touch the gather path: per-chunk diag(dis^2) matmuls on
  locally available features (L1: own x rows shipped per-core; L2: own relu1
  kept in SBUF).
- Band-major table layout (node (r,i) -> row (i//BND)*GRP + r*BND + i%BND)
  makes AllGather stage b fill exactly source group b, so 4 staged AllGathers
  (separate DRAM tensors for precise dependency tracking) pipeline with L1's
  transform, and L2 group-g gathers are gated only on stage g. x is
  pre-permuted host-side so both layers share one edge dataset.
- Per-chunk transform: cast, PE-transpose, matmul with W (+bias via a rank-1
  ones matmul), relu. Mean-pool via precomputed batch one-hot matmuls into
  per-core partials + AllReduce; MLP head computed redundantly on every core;
  core 0's output returned.
"""
import sys
sys.path.insert(0, '/opt/trn_rl_repo')
import contextlib
import numpy as np
import ml_dtypes

import concourse.bass as bass
import concourse.bacc as bacc
import concourse.mybir as mybir
import concourse.tile as tile
from concourse import library_config
from concourse.bass_utils import run_bass_kernel_spmd

BF16 = ml_dtypes.bfloat16
CORES = 8
F = 128          # feature/hidden width (fixed at 128 = partition width)
NGRP = 4         # src groups (int16 gather index limit)
CALL_TILES = 80  # tiles (of 128 rows) per dma_gather call


class Geom:
    def __init__(self, n_nodes=100000, n_edges=1600000, n_graphs=64, a_dim=8):
        assert n_nodes % (CORES * NGRP) == 0 or True
        self.N = n_nodes
        self.E = n_edges
        self.G = n_graphs
        self.A = a_dim
        self.NLOC = n_nodes // CORES
        self.GRP = n_nodes // NGRP
        assert self.GRP <= 32767, "int16 gather index limit"
        self.CH = (self.NLOC + 127) // 128  # dst chunks per core


def _prep(geom, x, edge_index, batch, W1, b1, W2, b2, fc1_w, fc1_b, fc2_w, fc2_b):
    """Host-side preprocessing: degrees, edge sharding/sorting, padding plan,
    per-core input arrays."""
    g_ = geom
    N, NLOC, GRP, CH = g_.N, g_.NLOC, g_.GRP, g_.CH
    src = np.asarray(edge_index[0], dtype=np.int64)
    dst = np.asarray(edge_index[1], dtype=np.int64)
    batch = np.asarray(batch, dtype=np.int64)

    deg = np.bincount(dst, minlength=N).astype(np.float32) + 1.0
    dis = (1.0 / np.sqrt(deg)).astype(np.float32)

    assert NLOC % NGRP == 0
    BND = NLOC // NGRP
    # band-major table layout: node u=(r,i) -> row (i//BND)*GRP + r*BND + i%BND
    # so AllGather stage b fills exactly table rows [b*GRP,(b+1)*GRP) = group b
    u = np.arange(N, dtype=np.int64)
    r_, i_ = u // NLOC, u % NLOC
    row_of_node = (i_ // BND) * GRP + r_ * BND + (i_ % BND)
    node_of_row = np.empty(N, np.int64)
    node_of_row[row_of_node] = u
    # (row_of_node is recomputed after balancing permutations below)

    core_of = dst // NLOC
    per_core = []
    core_posn = []  # per core: local node -> position (chunk*128+slot)
    cnt = np.zeros((CORES, NGRP * CH), np.int64)
    vloc = np.arange(NLOC, dtype=np.int64)
    for c in range(CORES):
        m = core_of == c
        s = src[m]
        d_raw = dst[m] - c * NLOC
        w = (dis[s] * dis[dst[m]]).astype(np.float32)  # dis_src*dis_dst
        s = row_of_node[s]  # table rows, band-major
        sg = s // GRP

        # balance per-(group,chunk) edge counts across cores by permuting
        # local nodes WITHIN their AllGather band (keeps src groups fixed)
        dvec = np.zeros((NLOC, NGRP), np.int64)
        np.add.at(dvec, (d_raw, sg), 1)
        posn = np.empty(NLOC, np.int64)
        Lb = np.zeros((CH, NGRP), np.float64)
        for b in range(NGRP):
            lo_n, hi_n = b * BND, (b + 1) * BND
            nodes = np.arange(lo_n, hi_n)
            nodes = nodes[np.argsort(-dvec[nodes].sum(1), kind='stable')]
            ch_lo, ch_hi = lo_n // 128, (hi_n - 1) // 128
            chs = np.arange(ch_lo, ch_hi + 1)
            cap = np.minimum((chs + 1) * 128, hi_n) - np.maximum(chs * 128, lo_n)
            nxt = np.maximum(chs * 128, lo_n).astype(np.int64)  # next free position
            left = cap.copy()
            for v in nodes:
                dots = Lb[chs] @ dvec[v]
                dots[left <= 0] = np.inf
                j = int(np.argmin(dots))
                posn[v] = nxt[j]
                nxt[j] += 1
                left[j] -= 1
                Lb[chs[j]] += dvec[v]
        core_posn.append(posn)

        d = posn[d_raw]
        ch = d >> 7
        sl = (d & 127).astype(np.int64)
        seg = sg * CH + ch
        order = np.argsort(seg, kind='stable')
        per_core.append((s[order], seg[order], sl[order], w[order]))
        cnt[c] = np.bincount(seg, minlength=NGRP * CH)

    # fold the balancing permutations into the table-row map; bands are
    # preserved by construction so src groups (seg) are unchanged
    i2 = np.concatenate(core_posn)  # [N] balanced local position per node
    row_of_node = (i2 // BND) * GRP + (u // NLOC) * BND + (i2 % BND)
    node_of_row = np.full(N, -1, np.int64)
    node_of_row[row_of_node] = u
    # remap each core's src table rows with the final permutation-aware map
    for c in range(CORES):
        s, seg, sl, w = per_core[c]
        m = core_of == c
        s_nodes = src[m]
        d_raw = dst[m] - c * NLOC
        sg = row_of_node[s_nodes] // GRP
        d = core_posn[c][d_raw]
        ch = d >> 7
        sl2 = (d & 127).astype(np.int64)
        seg2 = sg * CH + ch
        order = np.argsort(seg2, kind='stable')
        w2 = (dis[s_nodes] * dis[dst[m]]).astype(np.float32)
        per_core[c] = (row_of_node[s_nodes][order], seg2[order], sl2[order], w2[order])
        cnt[c] = np.bincount(seg2, minlength=NGRP * CH)

    L = cnt.max(axis=0)  # per-segment padded length = max count across cores
    # group streams rounded up to x128 via a tail filler segment
    base = np.zeros(NGRP * CH + 1, np.int64)
    grp_len = []
    off = 0
    for gidx in range(NGRP):
        for ch in range(CH):
            base[gidx * CH + ch] = off
            off += int(L[gidx * CH + ch])
        if off % 128:
            off += 128 - off % 128  # group tail pad (no pieces)
        grp_len.append(off - (int(base[gidx * CH])))
    base[-1] = off
    S_total = off
    grp_tiles = [gl // 128 for gl in grp_len]
    # fix base sentinel per group end (used only for slicing idx streams)
    grp_lo = [int(base[g * CH]) for g in range(NGRP)]

    # piece-level metadata (identical across cores): a piece is the part of a
    # segment falling in one 128-row tile
    piece_tile = []   # tile index within the group
    piece_chunk = []
    piece_first = []
    piece_last = []
    pieces_by_grp = []
    for gidx in range(NGRP):
        plist = []
        for ch in range(CH):
            segi = gidx * CH + ch
            lo = int(base[segi]) - grp_lo[gidx]
            hi = lo + int(L[segi])
            if hi == lo:
                continue
            tlo, thi = lo // 128, (hi - 1) // 128
            for t in range(tlo, thi + 1):
                plist.append((t, ch, t == tlo, t == thi))
        pieces_by_grp.append(plist)
        for (t, ch, fi, la) in plist:
            piece_tile.append(t)
            piece_chunk.append(ch)
            piece_first.append(fi)
            piece_last.append(la)
    NP_TOT = len(piece_tile)
    TT = S_total // 128

    # call plan per group: list of (ntiles, npieces); pieces are consumed in
    # order and each call covers the pieces whose tile is inside the call
    call_plan = []
    for gidx in range(NGRP):
        plist = pieces_by_grp[gidx]
        calls = []
        t0 = 0
        pi = 0
        left = grp_tiles[gidx]
        while left > 0:
            take = min(CALL_TILES, left)
            np_call = 0
            while pi < len(plist) and plist[pi][0] < t0 + take:
                np_call += 1
                pi += 1
            calls.append((take, np_call))
            t0 += take
            left -= take
        assert pi == len(plist)
        call_plan.append(calls)

    # per-core streams
    in_maps = []
    counts = np.bincount(batch, minlength=g_.G).astype(np.float32)
    invc = (1.0 / np.maximum(counts, 1.0)).astype(np.float32).reshape(g_.G, 1)
    xt = np.asarray(x, dtype=np.float32).astype(BF16)[node_of_row]  # band-major rows
    pad_nodes = CH * 128 - NLOC
    for c in range(CORES):
        s, seg, sl, w = per_core[c]
        # destination positions in padded stream
        seg_start_in_sorted = np.searchsorted(seg, np.arange(NGRP * CH))
        rank = np.arange(len(seg)) - seg_start_in_sorted[seg]
        pos = base[seg] + rank
        idxv = np.zeros(S_total, np.int16)
        idxv[pos] = (s - (s // GRP) * GRP).astype(np.int16)
        im = {}
        for gidx in range(NGRP):
            lo = grp_lo[gidx]
            hi = lo + grp_len[gidx]
            seg16 = idxv[lo:hi].reshape(-1, 16).T  # [16, n/16]
            im[f"idxg{gidx}"] = np.tile(seg16, (8, 1)).copy()
        # per-piece weighted one-hot tiles: A[p, e, d] = norm weight.
        # piece id for an edge: map (segment, tile-of-pos) -> piece index
        pk = {}
        for p, (gidx_, t_, ch_, fi_, la_) in enumerate(
                [(gg, t, ch, fi, la) for gg in range(NGRP)
                 for (t, ch, fi, la) in pieces_by_grp[gg]]):
            pk[(gidx_, t_, ch_)] = p
        e_g = seg // CH
        e_ch = seg % CH
        e_t = np.empty(len(pos), np.int64)
        for gidx in range(NGRP):
            m2 = e_g == gidx
            e_t[m2] = (pos[m2] - grp_lo[gidx]) // 128
        e_p = np.array([pk[(gg, tt_, cc)] for gg, tt_, cc in
                        zip(e_g, e_t, e_ch)], np.int64)
        A = np.zeros(NP_TOT * 128 * 128, BF16)
        e_slot = np.empty(len(pos), np.int64)
        for gidx in range(NGRP):
            m2 = e_g == gidx
            e_slot[m2] = (pos[m2] - grp_lo[gidx]) % 128
        A[e_p * (128 * 128) + e_slot * 128 + sl] = w.astype(BF16)
        im["oh"] = np.ascontiguousarray(
            A.reshape(NP_TOT, 128, 128).transpose(1, 0, 2).reshape(128, NP_TOT * 128))
        # precomputed batch one-hots: B[ch, node_slot, graph]
        B = np.zeros(CH * 128 * g_.G, BF16)
        bl = batch[c * NLOC:(c + 1) * NLOC]
        B[core_posn[c] * g_.G + bl] = np.float32(1.0)
        im["ohb"] = np.ascontiguousarray(
            B.reshape(CH, 128, g_.G).transpose(1, 0, 2).reshape(128, CH * g_.G))
        im["invc"] = invc
        im["xt"] = xt
        im["ident"] = np.eye(128, dtype=np.float32).astype(BF16)
        # self-loop handling without gathers: per-chunk diag(dis^2) + own x rows
        posn = core_posn[c]
        dis2 = np.zeros(CH * 128, np.float32)
        dis2[posn] = dis[c * NLOC:(c + 1) * NLOC] ** 2
        S2 = np.zeros((128, CH * 128), np.float32)
        nn = np.arange(CH * 128)
        S2[nn % 128, nn] = dis2
        im["selfoh"] = S2.astype(BF16)
        xs_loc = np.zeros((CH * 128, F), BF16)
        xs_loc[posn] = np.asarray(x, np.float32)[c * NLOC:(c + 1) * NLOC].astype(BF16)
        im["xself"] = xs_loc
        im["w1"] = np.asarray(W1, np.float32).astype(BF16)
        im["w2"] = np.asarray(W2, np.float32).astype(BF16)
        im["fc1w"] = np.asarray(fc1_w, np.float32).astype(BF16)
        im["fc2w"] = np.asarray(fc2_w, np.float32).astype(BF16)
        im["b1"] = np.asarray(b1, np.float32).astype(BF16).reshape(1, F)
        im["b2"] = np.asarray(b2, np.float32).astype(BF16).reshape(1, F)
        im["fc2b"] = np.asarray(fc2_b, np.float32).astype(BF16).reshape(1, g_.A)
        im["fc1b"] = np.asarray(fc1_b, np.float32).reshape(F, 1).copy()
        in_maps.append(im)

    plan = dict(
        TT=TT, NP_TOT=NP_TOT, grp_tiles=grp_tiles, call_plan=call_plan,
        piece_tile=piece_tile, piece_chunk=piece_chunk,
        piece_first=piece_first, piece_last=piece_last,
    )
    return plan, in_maps


def _build(geom, plan, tag="", stages="all"):
    g_ = geom
    N, NLOC, GRP, CH, G, A = g_.N, g_.NLOC, g_.GRP, g_.CH, g_.G, g_.A
    TT = plan["TT"]
    bf = mybir.dt.bfloat16
    f32 = mybir.dt.float32
    AL = mybir.AluOpType
    ACT = mybir.ActivationFunctionType

    nc = bacc.Bacc("TRN2", debug=False, target_bir_lowering=False)
    P = {}
    def par(name, shape, dt):
        P[name] = nc.declare_dram_parameter(name + tag, list(shape), dt, isOutput=False)
        return P[name]

    for gidx in range(NGRP):
        par(f"idxg{gidx}", [128, plan["grp_tiles"][gidx] * 8], mybir.dt.int16)
    par("oh", [128, plan["NP_TOT"] * 128], bf)
    par("ohb", [128, CH * G], bf)
    par("invc", [G, 1], f32)
    par("xt", [N, F], bf)
    par("ident", [128, 128], bf)
    par("selfoh", [128, CH * 128], bf)
    par("xself", [CH * 128, F], bf)
    par("w1", [F, F], bf)
    par("w2", [F, F], bf)
    par("fc1w", [F, F], bf)
    par("fc2w", [F, A], bf)
    par("b1", [1, F], bf)
    par("b2", [1, F], bf)
    par("fc2b", [1, A], bf)
    par("fc1b", [F, 1], f32)
    out_ext = nc.declare_dram_parameter("out" + tag, [G, A], f32, isOutput=True)

    BND = NLOC // NGRP
    agin = [nc.dram_tensor(f"agin{b}" + tag, [BND, F], bf) for b in range(NGRP)]
    tbl2 = [nc.dram_tensor(f"tbl2{b}" + tag, [GRP, F], bf, addr_space="Shared")
            for b in range(NGRP)]
    ar_in = nc.dram_tensor("arin" + tag, [G, F], f32)
    ar_out = nc.dram_tensor("arout" + tag, [G, F], f32, addr_space="Shared")

    with tile.TileContext(nc) as tc:
        with contextlib.ExitStack() as ex:
            pc = ex.enter_context(tc.tile_pool(name="const", bufs=1))
            pacc_pool = ex.enter_context(tc.tile_pool(name="accp", bufs=1))
            pidx = ex.enter_context(tc.tile_pool(name="idx", bufs=2))
            pg = ex.enter_context(tc.tile_pool(name="gbuf", bufs=2))
            poh = ex.enter_context(tc.tile_pool(name="oh", bufs=2))
            ptf = ex.enter_context(tc.tile_pool(name="tf", bufs=3))
            pseg = ex.enter_context(tc.tile_pool(name="pseg", bufs=2, space=bass.MemorySpace.PSUM))
            ptp = ex.enter_context(tc.tile_pool(name="ptp", bufs=6, space=bass.MemorySpace.PSUM))

            nc.gpsimd.load_library(library_config.mlp)

            # load constants
            ct = {}
            for nm in ["ohb", "ident",
                       "w1", "w2", "fc1w", "fc2w", "b1", "b2", "fc2b", "fc1b", "invc"]:
                t = pc.tile([P[nm].shape[0], P[nm].shape[1]], P[nm].dtype, tag=nm)
                nc.sync.dma_start(out=t[:], in_=P[nm][:, :])
                ct[nm] = t
            ones = pc.tile([1, 128], bf)
            nc.vector.memset(ones[:], 1.0)

            acc = pacc_pool.tile([128, CH * 128], f32)
            pacc = pacc_pool.tile([G, F], f32)
            rel1k = pacc_pool.tile([128, CH * 128], bf)


            layers = () if stages == "noop" else ((1,) if stages in ("edge1", "l1", "l1noag") else (1, 2))
            for layer in layers:
                wt = ct["w1"] if layer == 1 else ct["w2"]
                bt = ct["b1"] if layer == 1 else ct["b2"]

                # -------- edge phase --------
                p_global = 0
                ps = None
                for gidx in range(NGRP):
                    pos16 = 0  # column offset into idxg (16-wrapped)
                    t0call = 0
                    for (ntiles, npieces) in plan["call_plan"][gidx]:
                        nidx = ntiles * 128
                        idx_t = pidx.tile([128, nidx // 16], mybir.dt.int16)
                        nc.sync.dma_start(
                            out=idx_t[:],
                            in_=P[f"idxg{gidx}"][:, pos16:pos16 + nidx // 16])
                        gbuf = pg.tile([128, ntiles, F], bf)
                        srcap = (P["xt"].ap()[gidx * GRP:(gidx + 1) * GRP]
                                 if layer == 1 else tbl2[gidx].ap())
                        nc.gpsimd.dma_gather(
                            gbuf[:], srcap,
                            idx_t[:], nidx, nidx, F, single_packet=False)
                        ohslab = poh.tile([128, max(npieces, 1), 128], bf)
                        if npieces:
                            nc.sync.dma_start(
                                out=ohslab[:, :npieces, :],
                                in_=P["oh"].ap().rearrange("p (t d) -> p t d", d=128)[:, p_global:p_global + npieces, :])
                        for pp in range(npieces):
                            p = p_global + pp
                            ch = int(plan["piece_chunk"][p])
                            tloc = int(plan["piece_tile"][p]) - t0call
                            if plan["piece_first"][p]:
                                ps = pseg.tile([128, 128], f32)
                            nc.tensor.matmul(ps[:], ohslab[:, pp, :],
                                             gbuf[:, tloc, :],
                                             start=bool(plan["piece_first"][p]),
                                             stop=bool(plan["piece_last"][p]))
                            if plan["piece_last"][p]:
                                csl = acc[:, ch * 128:(ch + 1) * 128]
                                if gidx == 0:
                                    nc.vector.tensor_copy(csl, ps[:])
                                else:
                                    nc.vector.tensor_tensor(csl, csl, ps[:], AL.add)
                        p_global += npieces
                        t0call += ntiles
                        pos16 += nidx // 16

                # -------- transform phase --------
                ag_next = 0
                for ch in (range(CH) if stages != "edge1" else range(0)):
                    rows = min(128, NLOC - ch * 128)
                    # self-loop term: diag(dis^2) @ own features (no gather)
                    so = ptf.tile([128, 128], bf, tag="so")
                    nc.sync.dma_start(out=so[:], in_=P["selfoh"][:, ch * 128:(ch + 1) * 128])
                    if layer == 1:
                        xs = ptf.tile([128, 128], bf, tag="xs")
                        nc.sync.dma_start(out=xs[:], in_=P["xself"][ch * 128:(ch + 1) * 128, :])
                        selfrhs = xs[:]
                    else:
                        selfrhs = rel1k[:, ch * 128:(ch + 1) * 128]
                    ps2 = ptp.tile([128, 128], f32, tag="ps")
                    nc.tensor.matmul(ps2[:], so[:], selfrhs, start=True, stop=True)
                    csl2 = acc[:, ch * 128:(ch + 1) * 128]
                    nc.vector.tensor_tensor(csl2, csl2, ps2[:], AL.add)
                    aggS = ptf.tile([128, 128], bf)
                    nc.vector.tensor_copy(aggS[:], acc[:, ch * 128:(ch + 1) * 128])
                    psT = ptp.tile([128, 128], bf, tag="ps")
                    nc.tensor.transpose(psT[:], aggS[:], ct["ident"][:])
                    aggT = ptf.tile([128, 128], bf)
                    nc.scalar.copy(aggT[:], psT[:])
                    psO = ptp.tile([128, 128], f32, tag="ps")
                    nc.tensor.matmul(psO[:], aggT[:], wt[:], start=True, stop=False)
                    nc.tensor.matmul(psO[:], ones[:1, :], bt[:1, :], start=False, stop=True)
                    if layer == 1:
                        rel = rel1k[:, ch * 128:(ch + 1) * 128]
                    else:
                        rel_t = ptf.tile([128, 128], bf, tag="rel")
                        rel = rel_t[:]
                    nc.scalar.activation(rel, psO[:], ACT.Relu)
                    if layer == 1:
                        lo = ch * 128
                        hi = lo + rows
                        b0, b1 = lo // BND, (hi - 1) // BND
                        for b in range(b0, b1 + 1):
                            s0, s1 = max(lo, b * BND), min(hi, (b + 1) * BND)
                            nc.sync.dma_start(
                                out=agin[b][s0 - b * BND:s1 - b * BND, :],
                                in_=rel[s0 - lo:s1 - lo, :])
                        if stages not in ("edge1", "l1noag"):
                            while ag_next < NGRP and (ag_next + 1) * BND <= hi:
                                nc.gpsimd.collective_compute(
                                    "AllGather", AL.bypass,
                                    ins=[agin[ag_next].ap().opt()],
                                    outs=[tbl2[ag_next].ap().opt()],
                                    replica_groups=[list(range(CORES))])
                                ag_next += 1
                    else:
                        psB = ptp.tile([G, F], f32, tag="ps")
                        nc.tensor.matmul(psB[:], ct["ohb"][:, ch * G:(ch + 1) * G],
                                         rel, start=True, stop=True)
                        if ch == 0:
                            nc.vector.tensor_copy(pacc[:], psB[:])
                        else:
                            nc.vector.tensor_tensor(pacc[:], pacc[:], psB[:], AL.add)

                if layer == 1 and stages not in ("edge1", "l1noag"):
                    while ag_next < NGRP:
                        nc.gpsimd.collective_compute(
                            "AllGather", AL.bypass,
                            ins=[agin[ag_next].ap().opt()],
                            outs=[tbl2[ag_next].ap().opt()],
                            replica_groups=[list(range(CORES))])
                        ag_next += 1

            # -------- pooling + head --------
            if stages in ("edge1", "l1", "l1noag", "noop"):
                z0 = ptf.tile([G, A], f32)
                nc.vector.memset(z0[:], 0.0)
                nc.sync.dma_start(out=out_ext[:, :], in_=z0[:])
            else:
                nc.sync.dma_start(out=ar_in[:, :], in_=pacc[:])
                nc.gpsimd.collective_compute(
                    "AllReduce", AL.add,
                    ins=[ar_in.ap().opt()], outs=[ar_out.ap().opt()],
                    replica_groups=[list(range(CORES))])
                pooledf = ptf.tile([G, F], f32)
                nc.sync.dma_start(out=pooledf[:], in_=ar_out[:, :])
                pooledb = ptf.tile([G, F], bf)
                nc.vector.tensor_scalar(pooledb[:], pooledf[:], ct["invc"][:, :1],
                                        None, AL.mult)
                psPT = ptp.tile([F, G], bf, tag="ps")
                nc.tensor.transpose(psPT[:], pooledb[:], ct["ident"][:G, :G])
                pooledT = ptf.tile([F, G], bf)
                nc.scalar.copy(pooledT[:], psPT[:])
                psZ = ptp.tile([F, G], f32, tag="ps")
                nc.tensor.matmul(psZ[:], ct["fc1w"][:], pooledT[:], start=True, stop=True)
                zT = ptf.tile([F, G], bf)
                nc.scalar.activation(zT[:], psZ[:], ACT.Relu, bias=ct["fc1b"][:, :1])
                psO2 = ptp.tile([G, A], f32, tag="ps")
                nc.tensor.matmul(psO2[:], zT[:], ct["fc2w"][:], start=True, stop=False)
                nc.tensor.matmul(psO2[:], ones[:1, :G], ct["fc2b"][:1, :],
                                 start=False, stop=True)
                outt = ptf.tile([G, A], f32)
                nc.scalar.activation(outt[:], psO2[:], ACT.Sigmoid)
                nc.sync.dma_start(out=out_ext[:, :], in_=outt[:])

    nc.compile()
    return nc


_GEOM = Geom()
_CALLS = [0]


def kernel(x, edge_index, batch, W1, b1, W2, b2, fc1_w, fc1_b, fc2_w, fc2_b):
    plan, in_maps = _prep(_GEOM, x, edge_index, batch, W1, b1, W2, b2,
                          fc1_w, fc1_b, fc2_w, fc2_b)
    tag = f"_k{_CALLS[0]}" if _CALLS[0] else ""
    _CALLS[0] += 1
    nc = _build(_GEOM, plan, tag=tag)
    res = run_bass_kernel_spmd(nc, [{k + tag: v for k, v in m.items()} for m in in_maps],
                               list(range(CORES)))
    return np.asarray(res.results[0]["out" if not tag else "out" + tag],
                      dtype=np.float32)
